# revision 18
# baseline (speedup 1.0000x reference)
"""Trainium2 Bass kernel for nn_DataEmbedding_FeaturePatching.

Pure data-parallel over 8 NeuronCores: each core processes 128 of the
1024 fused (B*N) sequences end-to-end (conv branches -> magnitude pool
-> patchify -> fc1+gelu -> fc2 + positional encoding).

Token order inside a core is patch-major: token t = p*128 + s where
p = patch index (0..62) and s = local sequence (0..127).
"""

import numpy as np
from contextlib import ExitStack

import concourse.bacc as bacc
import concourse.bass as bass
import concourse.mybir as mybir
import concourse.tile as tile
from concourse import bass_utils, masks
import bass_rust

FP = mybir.dt.float32
AOP = mybir.AluOpType

B, L, NX, NM = 16, 512, 60, 4
N = NX + NM              # 64 fused channels
SEQS = B * N             # 1024
NCORES = 8
S = SEQS // NCORES       # 128 sequences per core
PATCH, STRIDE = 16, 8
NPATCH = 63
EMBED = 512
INNER = 160              # 10 ch * 16
H4 = 4 * EMBED           # 2048
MAXPAD = 14
XPW = L + MAXPAD         # padded input width

# (pad_rep, k_conv, pad_zero, k_pool)
BRANCHES = [(4, 5, 2, 3), (8, 9, 4, 5), (14, 15, 6, 7)]

_CACHED_NC = None
LAST_RESULTS = None


def _wcol(br, c, j):
    """Column of wconv holding conv weight w[br][c, 0, j]."""
    base = [0, 15, 42][br]
    k = BRANCHES[br][1]
    return base + c * k + j


def _bcol(br, c):
    """Column of wconv holding conv bias b[br][c]."""
    return 87 + br * 3 + c


def _custom_ap(ap, offset_add, dims):
    a = ap.copy()
    a.offset = a.offset + offset_add
    a.ap = bass_rust.VecI64Pair(dims)
    return a


def _declare_io(nc):
    return dict(
        x_in=nc.dram_tensor("x_in", [S, L], FP, kind="ExternalInput"),
        wconv=nc.dram_tensor("wconv", [128, 96], FP, kind="ExternalInput"),
        fc1wT=nc.dram_tensor("fc1wT", [INNER, H4], FP, kind="ExternalInput"),
        fc1b=nc.dram_tensor("fc1b", [H4], FP, kind="ExternalInput"),
        fc2wT=nc.dram_tensor("fc2wT", [H4, EMBED], FP, kind="ExternalInput"),
        bias2=nc.dram_tensor("bias2", [NPATCH, EMBED], FP, kind="ExternalInput"),
        embx=nc.dram_tensor("embx", [S, EMBED], FP, kind="ExternalInput"),
        out=nc.dram_tensor("out", [NPATCH, S, EMBED], FP, kind="ExternalOutput"),
        featT=nc.dram_tensor("featT", [10, L, S], FP),  # internal scratch
    )


def _kernel_body(tc, ctx, io, stage=99):
    nc = tc.nc
    x_in = io["x_in"]; wconv = io["wconv"]; fc1wT = io["fc1wT"]
    fc1b = io["fc1b"]; fc2wT = io["fc2wT"]; bias2 = io["bias2"]
    embx = io["embx"]; out = io["out"]; featT = io["featT"]

    const = ctx.enter_context(tc.tile_pool(name="const", bufs=1))

    wc = const.tile([128, 96], FP, tag="wc")
    nc.sync.dma_start(wc[:], wconv.ap())

    ident = const.tile([128, 128], FP, tag="ident")
    masks.make_identity(nc, ident[:])

    ones = const.tile([1, 128], FP, tag="ones")
    nc.vector.memset(ones[:], 1.0)

    embx_sb = const.tile([128, EMBED], FP, tag="embx")
    nc.sync.dma_start(embx_sb[:], embx.ap())

    fc1wa = const.tile([128, H4], FP, tag="fc1wa")
    nc.sync.dma_start(fc1wa[:], fc1wT.ap()[0:128, :])
    fc1wb = const.tile([32, H4], FP, tag="fc1wb")
    nc.sync.dma_start(fc1wb[:], fc1wT.ap()[128:INNER, :])

    # fc1 bias laid out [128 part, 16 chunks]: element (p, m) <- fc1b[m*128 + p]
    fc1b_sb = const.tile([128, 16], FP, tag="fc1b")
    nc.sync.dma_start(fc1b_sb[:], _custom_ap(fc1b.ap(), 0, [[1, 128], [128, 16]]))

    # fc2 weightsT [128 part, (chunk 16, o 512)]: (p, c, o) <- fc2wT[c*128+p, o]
    w2 = const.tile([128, 16 * EMBED], FP, tag="w2")
    nc.sync.dma_start(
        w2[:], _custom_ap(fc2wT.ap(), 0, [[EMBED, 128], [128 * EMBED, 16], [1, EMBED]])
    )



    # ---- Phase A: conv branches + magnitude pooling --------------------
    xpad = const.tile([128, XPW], FP, tag="xpad")
    nc.sync.dma_start(xpad[:, MAXPAD:XPW], x_in.ap())
    # left edge-pad: replicate column 0 of x into cols 0..13
    nc.vector.tensor_copy(xpad[:, 0:MAXPAD], xpad[:, MAXPAD : MAXPAD + 1].broadcast_to((128, MAXPAD)))

    feat_pool = ctx.enter_context(tc.tile_pool(name="feat", bufs=1))
    work = ctx.enter_context(tc.tile_pool(name="convwork", bufs=2))

    feats = []  # 9 conv feature planes, in channel order x1c0..x3c2
    for br, (pad_rep, kc, pz, kp) in enumerate(BRANCHES):
        for c in range(3):
            ypad = work.tile([128, L + pz], FP, tag="ypad")
            nc.vector.memset(ypad[:, 0:pz], 0.0)
            acc = ypad[:, pz : pz + L]
            # conv: acc[t] = sum_j w[c,j] * xpad[t + (MAXPAD-pad_rep) + j]
            off = MAXPAD - pad_rep
            nc.vector.tensor_scalar(
                acc, xpad[:, off : off + L], wc[:, _wcol(br, c, 0) : _wcol(br, c, 0) + 1],
                None, op0=AOP.mult,
            )
            for j in range(1, kc):
                nc.vector.scalar_tensor_tensor(
                    acc, xpad[:, off + j : off + j + L],
                    wc[:, _wcol(br, c, j) : _wcol(br, c, j) + 1],
                    acc, op0=AOP.mult, op1=AOP.add,
                )
            nc.vector.tensor_scalar_add(acc, acc, wc[:, _bcol(br, c) : _bcol(br, c) + 1])

            # magnitude max-pool over kp windows, stride 1, first-max ties
            U32 = mybir.dt.uint32
            best = feat_pool.tile([128, L], FP, tag=f"f{br}_{c}")
            babs = work.tile([128, L], FP, tag="babs")
            cab = work.tile([128, L], FP, tag="cab")
            mask = work.tile([128, L], mybir.dt.uint8, tag="mask")
            nc.vector.tensor_copy(best[:], ypad[:, 0:L])
            nc.vector.tensor_scalar(
                babs[:].bitcast(U32), ypad[:, 0:L].bitcast(U32),
                0x7FFFFFFF, None, op0=AOP.bitwise_and,
            )
            for j in range(1, kp):
                cand = ypad[:, j : j + L]
                nc.vector.tensor_scalar(
                    cab[:].bitcast(U32), cand.bitcast(U32),
                    0x7FFFFFFF, None, op0=AOP.bitwise_and,
                )
                nc.vector.tensor_tensor(mask[:], cab[:], babs[:], op=AOP.is_gt)
                nc.vector.copy_predicated(best[:], mask[:], cand)
                nc.vector.tensor_tensor(babs[:], cab[:], babs[:], op=AOP.max)
            feats.append(best)

    if stage <= 1:  # conv+pool only
        for i, f in enumerate(feats):
            nc.sync.dma_start(out.ap()[i], f[:])
        return

    # ---- transpose feature planes into DRAM featT [10, L, S] -----------
    with tc.tile_pool(name="tpsum", bufs=4, space="PSUM") as tpsum, \
         tc.tile_pool(name="tsb", bufs=4) as tsb:
        planes = [xpad[:, MAXPAD:XPW]] + [f[:] for f in feats]
        for ci, plane in enumerate(planes):
            for k in range(4):
                pt = tpsum.tile([128, 128], FP, tag="pt")
                nc.tensor.transpose(pt[:], plane[:, 128 * k : 128 * (k + 1)], ident[:])
                st = tsb.tile([128, 128], FP, tag="st")
                nc.vector.tensor_copy(st[:], pt[:])
                nc.sync.dma_start(featT.ap()[ci, 128 * k : 128 * (k + 1), :], st[:])

    if stage <= 2:  # + transposes: read featT back out
        with tc.tile_pool(name="dbg", bufs=2) as dbg:
            for i in range(10):
                t = dbg.tile([128, 512], FP, tag="t")
                nc.sync.dma_start(
                    t[:], _custom_ap(featT.ap(), i * L * S, [[1, 128], [128, 512]])
                )
                nc.sync.dma_start(out.ap()[i], t[:])
        return

    # ---- Phase B: fc1 + gelu + fc2 + positional encoding ---------------
    zt_pool = ctx.enter_context(tc.tile_pool(name="zt", bufs=3))
    h_pool = ctx.enter_context(tc.tile_pool(name="h", bufs=2))
    ps1 = ctx.enter_context(tc.tile_pool(name="ps1", bufs=4, space="PSUM"))
    ps2 = ctx.enter_context(tc.tile_pool(name="ps2", bufs=3, space="PSUM"))
    osb_pool = ctx.enter_context(tc.tile_pool(name="osb", bufs=3))

    fap = featT.ap()
    n_tiles = 16
    for ti in range(n_tiles):
        p0 = ti * 4
        npt = min(4, NPATCH - p0)
        T = npt * S

        # gather z^T tiles straight from featT with overlapped-window APs
        zta = zt_pool.tile([128, 512], FP, tag="a")
        ztb = zt_pool.tile([32, 512], FP, tag="b")
        for c in range(10):
            dst = zta if c < 8 else ztb
            pbase = (c % 8) * PATCH
            nc.sync.dma_start(
                dst[pbase : pbase + PATCH, :T],
                _custom_ap(fap, c * L * S + p0 * STRIDE * S,
                           [[S, PATCH], [STRIDE * S, npt], [1, S]]),
            )

        if stage <= 3:  # gather only
            if ti < 8:
                nc.sync.dma_start(out.ap()[ti], zta[:, :512])
            continue

        ht = h_pool.tile([128, 16 * 512], FP, tag="ht")
        for m in range(16):
            ps = ps1.tile([128, 512], FP, tag="ps")
            nc.tensor.matmul(ps[:, :T], fc1wa[:, m * 128 : (m + 1) * 128],
                             zta[:, :T], start=True, stop=False)
            nc.tensor.matmul(ps[:, :T], fc1wb[:, m * 128 : (m + 1) * 128],
                             ztb[:, :T], start=False, stop=True)
            nc.scalar.activation(
                ht[:, m * 512 : m * 512 + T], ps[:, :T],
                mybir.ActivationFunctionType.Gelu,
                bias=fc1b_sb[:, m : m + 1], scale=1.0,
            )

        if stage <= 4:  # fc1+gelu only
            if ti < 8:
                nc.sync.dma_start(out.ap()[ti], ht[:, 0:512])
            continue

        for sti in range(npt):
            q = p0 + sti
            b2q = zt_pool.tile([1, EMBED], FP, tag="b2q")
            nc.sync.dma_start(b2q[:], bias2.ap()[q])
            po = ps2.tile([128, EMBED], FP, tag="po")
            for c in range(16):
                nc.tensor.matmul(
                    po[:], ht[:, c * 512 + sti * 128 : c * 512 + sti * 128 + 128],
                    w2[:, c * EMBED : (c + 1) * EMBED],
                    start=(c == 0), stop=False,
                )
            nc.tensor.matmul(po[:], ones[:], b2q[:],
                             start=False, stop=True)
            osb = osb_pool.tile([128, EMBED], FP, tag="osb")
            nc.vector.tensor_add(osb[:], po[:], embx_sb[:])
            nc.sync.dma_start(out.ap()[q], osb[:])


def _build_nc(repeat=1):
    nc = bacc.Bacc("TRN2", target_bir_lowering=False, debug=False)
    with tile.TileContext(nc) as tc:
        io = _declare_io(nc)
        for _ in range(repeat):
            with ExitStack() as ctx:
                _kernel_body(tc, ctx, io)
    nc.compile()
    return nc


def _pe_tables():
    """emb_x/emb_y tables of the PositionalEncoding2D (ch=512)."""
    channels = 256
    inv_freq = 1.0 / (10000 ** (np.arange(0, channels, 2) / np.float32(channels)))
    inv_freq = inv_freq.astype(np.float32)

    def emb1d(n):
        s = np.arange(n, dtype=np.float32)[:, None] * inv_freq[None, :]
        return np.stack([np.sin(s), np.cos(s)], axis=-1).reshape(n, -1)

    return emb1d(N).astype(np.float32), emb1d(NPATCH).astype(np.float32)


def _prepare_in_maps(x, x_mark, w1, b1, w2, b2, w3, b3, fc1_w, fc1_b, fc2_w, fc2_b):
    x = np.asarray(x, np.float32)
    x_mark = np.asarray(x_mark, np.float32)

    # fused sequences (b, n): n<60 from x, n>=60 from x_mark
    xc = np.concatenate(
        [np.ascontiguousarray(x.transpose(0, 2, 1)),
         np.ascontiguousarray(x_mark.transpose(0, 2, 1))], axis=1
    ).reshape(SEQS, L)

    wconv = np.concatenate([
        np.asarray(w1, np.float32).reshape(-1),
        np.asarray(w2, np.float32).reshape(-1),
        np.asarray(w3, np.float32).reshape(-1),
        np.asarray(b1, np.float32).reshape(-1),
        np.asarray(b2, np.float32).reshape(-1),
        np.asarray(b3, np.float32).reshape(-1),
    ])
    wconv = np.tile(wconv[None, :], (128, 1)).astype(np.float32)

    emb_x, emb_y = _pe_tables()
    # x-part of PE, expanded over the 128 local sequences (n = s % 64),
    # zero-padded over the y-channel half
    embx_full = np.zeros((S, EMBED), np.float32)
    embx_full[:, :256] = np.tile(emb_x, (S // N, 1))
    # y-part + fc2 bias folded together, per patch
    fc2_b = np.asarray(fc2_b, np.float32)
    bias2 = np.tile(fc2_b[None, :], (NPATCH, 1))
    bias2[:, 256:512] += emb_y

    shared = {
        "wconv": wconv,
        "fc1wT": np.ascontiguousarray(np.asarray(fc1_w, np.float32).T),
        "fc1b": np.asarray(fc1_b, np.float32),
        "fc2wT": np.ascontiguousarray(np.asarray(fc2_w, np.float32).T),
        "bias2": bias2,
        "embx": embx_full,
    }
    in_maps = []
    for k in range(NCORES):
        m = dict(shared)
        m["x_in"] = np.ascontiguousarray(xc[k * S : (k + 1) * S])
        in_maps.append(m)
    return in_maps


def kernel(x, x_mark, w1, b1, w2, b2, w3, b3, fc1_w, fc1_b, fc2_w, fc2_b):
    global _CACHED_NC, LAST_RESULTS
    if _CACHED_NC is None:
        _CACHED_NC = _build_nc()
    nc = _CACHED_NC

    in_maps = _prepare_in_maps(x, x_mark, w1, b1, w2, b2, w3, b3,
                               fc1_w, fc1_b, fc2_w, fc2_b)
    LAST_RESULTS = bass_utils.run_bass_kernel_spmd(
        nc, in_maps, core_ids=list(range(NCORES))
    )
    outs = np.stack([r["out"] for r in LAST_RESULTS.results])  # [8, 63, 128, 512]
    full = outs.transpose(0, 2, 1, 3).reshape(B, N * NPATCH, EMBED)
    return full


# revision 22
# speedup vs baseline: 3.5446x; 3.5446x over previous
"""Trainium2 Bass kernel for nn_DataEmbedding_FeaturePatching.

Pure data-parallel over 8 NeuronCores: each core processes 128 of the
1024 fused (B*N) sequences end-to-end (conv branches -> magnitude pool
-> patchify -> fc1+gelu -> fc2 + positional encoding).

Token order inside a core is patch-major: token t = p*128 + s where
p = patch index (0..62) and s = local sequence (0..127).
"""

import numpy as np
from contextlib import ExitStack

import concourse.bacc as bacc
import concourse.bass as bass
import concourse.mybir as mybir
import concourse.tile as tile
from concourse import bass_utils, masks
import bass_rust

FP = mybir.dt.float32
FPR = mybir.dt.float32r
AOP = mybir.AluOpType

B, L, NX, NM = 16, 512, 60, 4
N = NX + NM              # 64 fused channels
SEQS = B * N             # 1024
NCORES = 8
S = SEQS // NCORES       # 128 sequences per core
PATCH, STRIDE = 16, 8
NPATCH = 63
EMBED = 512
INNER = 160              # 10 ch * 16
H4 = 4 * EMBED           # 2048
MAXPAD = 14
XPW = L + MAXPAD         # padded input width

# (pad_rep, k_conv, pad_zero, k_pool)
BRANCHES = [(4, 5, 2, 3), (8, 9, 4, 5), (14, 15, 6, 7)]

_CACHED_NC = None
LAST_RESULTS = None


def _wcol(br, c, j):
    """Column of wconv holding conv weight w[br][c, 0, j]."""
    base = [0, 15, 42][br]
    k = BRANCHES[br][1]
    return base + c * k + j


def _bcol(br, c):
    """Column of wconv holding conv bias b[br][c]."""
    return 87 + br * 3 + c


def _custom_ap(ap, offset_add, dims):
    a = ap.copy()
    a.offset = a.offset + offset_add
    a.ap = bass_rust.VecI64Pair(dims)
    return a


def _declare_io(nc):
    return dict(
        x_in=nc.dram_tensor("x_in", [S, L], FP, kind="ExternalInput"),
        wconv=nc.dram_tensor("wconv", [128, 96], FP, kind="ExternalInput"),
        fc1wT=nc.dram_tensor("fc1wT", [INNER, H4], FP, kind="ExternalInput"),
        fc1b=nc.dram_tensor("fc1b", [H4], FP, kind="ExternalInput"),
        fc2wT=nc.dram_tensor("fc2wT", [H4, EMBED], FP, kind="ExternalInput"),
        bias2=nc.dram_tensor("bias2", [NPATCH, EMBED], FP, kind="ExternalInput"),
        embx=nc.dram_tensor("embx", [S, EMBED], FP, kind="ExternalInput"),
        ones_in=nc.dram_tensor("ones_in", [1, 128], FP, kind="ExternalInput"),
        out=nc.dram_tensor("out", [NPATCH, S, EMBED], FP, kind="ExternalOutput"),
        featT=nc.dram_tensor("featT", [10, L, S], FP),  # internal scratch
    )


def _kernel_body(tc, ctx, io, stage=99):
    nc = tc.nc
    x_in = io["x_in"]; wconv = io["wconv"]; fc1wT = io["fc1wT"]
    fc1b = io["fc1b"]; fc2wT = io["fc2wT"]; bias2 = io["bias2"]
    embx = io["embx"]; out = io["out"]; featT = io["featT"]

    const = ctx.enter_context(tc.tile_pool(name="const", bufs=1))

    wc = const.tile([128, 96], FP, tag="wc")
    nc.sync.dma_start(wc[:], wconv.ap())

    ident = const.tile([128, 128], FP, tag="ident")
    masks.make_identity(nc, ident[:])

    ones = const.tile([1, 128], FPR, tag="ones")
    nc.sync.dma_start(ones[:], io["ones_in"].ap().bitcast(FPR))

    embx_sb = const.tile([128, EMBED], FP, tag="embx")
    nc.sync.dma_start(embx_sb[:], embx.ap())

    fc1wa = const.tile([128, H4], FPR, tag="fc1wa")
    nc.sync.dma_start(fc1wa[:], fc1wT.ap()[0:128, :].bitcast(FPR))
    fc1wb = const.tile([32, H4], FPR, tag="fc1wb")
    nc.sync.dma_start(fc1wb[:], fc1wT.ap()[128:INNER, :].bitcast(FPR))

    # fc1 bias laid out [128 part, 16 chunks]: element (p, m) <- fc1b[m*128 + p]
    fc1b_sb = const.tile([128, 16], FP, tag="fc1b")
    nc.sync.dma_start(fc1b_sb[:], _custom_ap(fc1b.ap(), 0, [[1, 128], [128, 16]]))

    # fc2 weightsT [128 part, (chunk 16, o 512)]: (p, c, o) <- fc2wT[c*128+p, o]
    w2 = const.tile([128, 16 * EMBED], FPR, tag="w2")
    nc.sync.dma_start(
        w2[:],
        _custom_ap(fc2wT.ap(), 0,
                   [[EMBED, 128], [128 * EMBED, 16], [1, EMBED]]).bitcast(FPR),
    )



    # ---- Phase A: conv branches + magnitude pooling --------------------
    xpad = const.tile([128, XPW], FP, tag="xpad")
    nc.sync.dma_start(xpad[:, MAXPAD:XPW], x_in.ap())
    # left edge-pad: replicate column 0 of x into cols 0..13
    nc.vector.tensor_copy(xpad[:, 0:MAXPAD], xpad[:, MAXPAD : MAXPAD + 1].broadcast_to((128, MAXPAD)))

    feat_pool = ctx.enter_context(tc.tile_pool(name="feat", bufs=1))
    work = ctx.enter_context(tc.tile_pool(name="convwork", bufs=2))
    tpsum = ctx.enter_context(tc.tile_pool(name="tpsum", bufs=2, space="PSUM"))
    tsb = ctx.enter_context(tc.tile_pool(name="tsb", bufs=4))
    zt_pool = ctx.enter_context(tc.tile_pool(name="zt", bufs=3))
    h_pool = ctx.enter_context(tc.tile_pool(name="h", bufs=2))
    ps1 = ctx.enter_context(tc.tile_pool(name="ps1", bufs=4, space="PSUM"))
    ps2 = ctx.enter_context(tc.tile_pool(name="ps2", bufs=2, space="PSUM"))
    osb_pool = ctx.enter_context(tc.tile_pool(name="osb", bufs=3))

    U32 = mybir.dt.uint32
    chans = [(br, c) + tuple(BRANCHES[br]) for br in range(3) for c in range(3)]
    feats = [feat_pool.tile([128, L], FP, tag=f"f{i}") for i in range(9)]
    ypads = [feat_pool.tile([128, L + ch[4]], FP, tag=f"y{i}")
             for i, ch in enumerate(chans)]
    fap = featT.ap()

    def conv_range(i, s0, e1):
        """conv outputs [s0, e1) for channel i into its ypad."""
        br, c, pad_rep, kc, pz, kp = chans[i]
        ypad = ypads[i]
        if s0 == 0:
            nc.vector.memset(ypad[:, 0:pz], 0.0)
        acc = ypad[:, pz + s0 : pz + e1]
        off = MAXPAD - pad_rep + s0
        W = e1 - s0
        nc.vector.tensor_scalar(
            acc, xpad[:, off : off + W], wc[:, _wcol(br, c, 0) : _wcol(br, c, 0) + 1],
            None, op0=AOP.mult,
        )
        for j in range(1, kc):
            nc.vector.scalar_tensor_tensor(
                acc, xpad[:, off + j : off + j + W],
                wc[:, _wcol(br, c, j) : _wcol(br, c, j) + 1],
                acc, op0=AOP.mult, op1=AOP.add,
            )
        nc.vector.tensor_scalar_add(acc, acc, wc[:, _bcol(br, c) : _bcol(br, c) + 1])

    def pool_range(i, l0, l1):
        """magnitude max-pool outputs [l0, l1) for channel i (first-max ties)."""
        br, c, pad_rep, kc, pz, kp = chans[i]
        ypad, best = ypads[i], feats[i]
        W = l1 - l0
        babs = work.tile([128, L], FP, tag="babs")
        cab = work.tile([128, L], FP, tag="cab")
        mask = work.tile([128, L], mybir.dt.uint8, tag="mask")
        nc.vector.tensor_copy(best[:, l0:l1], ypad[:, l0:l1])
        nc.vector.tensor_scalar(
            babs[:, :W].bitcast(U32),
            ypad[:, l0:l1].bitcast(U32), 0x7FFFFFFF, None, op0=AOP.bitwise_and,
        )
        for j in range(1, kp):
            cand = ypad[:, l0 + j : l1 + j]
            nc.vector.tensor_scalar(
                cab[:, :W].bitcast(U32), cand.bitcast(U32),
                0x7FFFFFFF, None, op0=AOP.bitwise_and,
            )
            nc.vector.tensor_tensor(mask[:, :W], cab[:, :W], babs[:, :W], op=AOP.is_gt)
            nc.vector.copy_predicated(best[:, l0:l1], mask[:, :W], cand)
            nc.vector.tensor_tensor(babs[:, :W], cab[:, :W], babs[:, :W], op=AOP.max)

    def transpose_block(ci, plane, k):
        pt = tpsum.tile([128, 128], FP, tag="pt")
        nc.tensor.transpose(pt[:], plane[:, 128 * k : 128 * (k + 1)], ident[:])
        st = tsb.tile([128, 128], FP, tag="st")
        nc.vector.tensor_copy(st[:], pt[:])
        nc.sync.dma_start(featT.ap()[ci, 128 * k : 128 * (k + 1), :], st[:])

    def fc_tile(ti):
        p0 = ti * 4
        npt = min(4, NPATCH - p0)
        T = npt * S

        # gather z^T tiles straight from featT with overlapped-window APs
        zta = zt_pool.tile([128, 512], FPR, tag="a")
        ztb = zt_pool.tile([32, 512], FPR, tag="b")
        for c in range(10):
            dst = zta if c < 8 else ztb
            pbase = (c % 8) * PATCH
            nc.sync.dma_start(
                dst[pbase : pbase + PATCH, :T],
                _custom_ap(fap, c * L * S + p0 * STRIDE * S,
                           [[S, PATCH], [STRIDE * S, npt], [1, S]]).bitcast(FPR),
            )

        ht = h_pool.tile([128, 16 * 512], FPR, tag="ht")
        for m in range(16):
            ps = ps1.tile([128, 512], FP, tag="ps")
            nc.tensor.matmul(ps[:, :T], fc1wa[:, m * 128 : (m + 1) * 128],
                             zta[:, :T], start=True, stop=False)
            nc.tensor.matmul(ps[:, :T], fc1wb[:, m * 128 : (m + 1) * 128],
                             ztb[:, :T], start=False, stop=True)
            nc.scalar.activation(
                ht[:, m * 512 : m * 512 + T], ps[:, :T],
                mybir.ActivationFunctionType.Gelu,
                bias=fc1b_sb[:, m : m + 1], scale=1.0,
            )

        for sti in range(npt):
            q = p0 + sti
            b2q = zt_pool.tile([1, EMBED], FPR, tag="b2q")
            nc.sync.dma_start(b2q[:], bias2.ap()[q].bitcast(FPR))
            po = ps2.tile([128, EMBED], FP, tag="po")
            for c in range(16):
                nc.tensor.matmul(
                    po[:], ht[:, c * 512 + sti * 128 : c * 512 + sti * 128 + 128],
                    w2[:, c * EMBED : (c + 1) * EMBED],
                    start=(c == 0), stop=False,
                )
            nc.tensor.matmul(po[:], ones[:], b2q[:],
                             start=False, stop=True)
            osb = osb_pool.tile([128, EMBED], FP, tag="osb")
            nc.vector.tensor_add(osb[:], po[:], embx_sb[:])
            nc.sync.dma_start(out.ap()[q], osb[:])

    # Two L-halves pipelined: conv/pool/transpose of half h, then its fc
    # tiles — the scheduler overlaps half-1 conv (DVE) with half-0 fc (PE).
    planes = [xpad[:, MAXPAD:XPW]] + [f[:] for f in feats]
    done_conv = [0] * 9
    HALF = L // 2
    for hi, (l0, l1) in enumerate([(0, HALF), (HALF, L)]):
        for i in range(9):
            kp, pz = chans[i][5], chans[i][4]
            e1 = min(L, l1 + kp - 1 - pz) if hi == 0 else L
            if e1 > done_conv[i]:
                conv_range(i, done_conv[i], e1)
                done_conv[i] = e1
            pool_range(i, l0, l1)
        for ci, plane in enumerate(planes):
            for k in (2 * hi, 2 * hi + 1):
                transpose_block(ci, plane, k)
        tiles = range(0, 7) if hi == 0 else range(7, 16)
        for ti in tiles:
            fc_tile(ti)


def _build_nc(repeat=1):
    nc = bacc.Bacc("TRN2", target_bir_lowering=False, debug=False)
    with tile.TileContext(nc) as tc:
        io = _declare_io(nc)
        for _ in range(repeat):
            with ExitStack() as ctx:
                _kernel_body(tc, ctx, io)
    nc.compile()
    return nc


def _pe_tables():
    """emb_x/emb_y tables of the PositionalEncoding2D (ch=512)."""
    channels = 256
    inv_freq = 1.0 / (10000 ** (np.arange(0, channels, 2) / np.float32(channels)))
    inv_freq = inv_freq.astype(np.float32)

    def emb1d(n):
        s = np.arange(n, dtype=np.float32)[:, None] * inv_freq[None, :]
        return np.stack([np.sin(s), np.cos(s)], axis=-1).reshape(n, -1)

    return emb1d(N).astype(np.float32), emb1d(NPATCH).astype(np.float32)


def _prepare_in_maps(x, x_mark, w1, b1, w2, b2, w3, b3, fc1_w, fc1_b, fc2_w, fc2_b):
    x = np.asarray(x, np.float32)
    x_mark = np.asarray(x_mark, np.float32)

    # fused sequences (b, n): n<60 from x, n>=60 from x_mark
    xc = np.concatenate(
        [np.ascontiguousarray(x.transpose(0, 2, 1)),
         np.ascontiguousarray(x_mark.transpose(0, 2, 1))], axis=1
    ).reshape(SEQS, L)

    wconv = np.concatenate([
        np.asarray(w1, np.float32).reshape(-1),
        np.asarray(w2, np.float32).reshape(-1),
        np.asarray(w3, np.float32).reshape(-1),
        np.asarray(b1, np.float32).reshape(-1),
        np.asarray(b2, np.float32).reshape(-1),
        np.asarray(b3, np.float32).reshape(-1),
    ])
    wconv = np.tile(wconv[None, :], (128, 1)).astype(np.float32)

    emb_x, emb_y = _pe_tables()
    # x-part of PE, expanded over the 128 local sequences (n = s % 64),
    # zero-padded over the y-channel half
    embx_full = np.zeros((S, EMBED), np.float32)
    embx_full[:, :256] = np.tile(emb_x, (S // N, 1))
    # y-part + fc2 bias folded together, per patch
    fc2_b = np.asarray(fc2_b, np.float32)
    bias2 = np.tile(fc2_b[None, :], (NPATCH, 1))
    bias2[:, 256:512] += emb_y

    shared = {
        "ones_in": np.ones((1, 128), np.float32),
        "wconv": wconv,
        "fc1wT": np.ascontiguousarray(np.asarray(fc1_w, np.float32).T),
        "fc1b": np.asarray(fc1_b, np.float32),
        "fc2wT": np.ascontiguousarray(np.asarray(fc2_w, np.float32).T),
        "bias2": bias2,
        "embx": embx_full,
    }
    in_maps = []
    for k in range(NCORES):
        m = dict(shared)
        m["x_in"] = np.ascontiguousarray(xc[k * S : (k + 1) * S])
        in_maps.append(m)
    return in_maps


def kernel(x, x_mark, w1, b1, w2, b2, w3, b3, fc1_w, fc1_b, fc2_w, fc2_b):
    global _CACHED_NC, LAST_RESULTS
    if _CACHED_NC is None:
        _CACHED_NC = _build_nc()
    nc = _CACHED_NC

    in_maps = _prepare_in_maps(x, x_mark, w1, b1, w2, b2, w3, b3,
                               fc1_w, fc1_b, fc2_w, fc2_b)
    LAST_RESULTS = bass_utils.run_bass_kernel_spmd(
        nc, in_maps, core_ids=list(range(NCORES))
    )
    outs = np.stack([r["out"] for r in LAST_RESULTS.results])  # [8, 63, 128, 512]
    full = outs.transpose(0, 2, 1, 3).reshape(B, N * NPATCH, EMBED)
    return full
